# revision 1
# baseline (speedup 1.0000x reference)
"""HMM scaled-forward (alpha scaling) kernel for Trainium2, 8 NeuronCores.

Math: alpha_t = normalize((alpha_{t-1} @ A) * b[:, x_t]).
The map v -> normalize((v @ A) * e) is a Hilbert-metric contraction (A is a
dense positive stochastic matrix; diagonal emission scaling is an isometry),
so the T=1M sequential scan is split into independent chains, each seeded by
a 32-step host-side warmup (converges to fp32 machine precision in ~16
steps). Per-step normalization is dropped on device (prescaled emissions
keep the unnormalized state within e^{+-10} over a chain); rows are
normalized on the host at the end.

Layout per core: GRP independent sub-batches (to pipeline PE<->DVE since
each sub-batch's recurrence is serial), each sub-batch packs 2x F chains
into 128 partitions (two 64-state groups, block-diag A).
Device per step and sub-batch: S = (S^T @ blockdiag(A,A)) * E (PE + DVE).
History is transposed per chain-pair on the PE into output-row layout,
copied PSUM->SBUF on ACT, and DMA'd out. Emissions are pre-gathered on the
host (TRN2 has no fast dynamic gather) and streamed in consumption order.
"""

import sys
import os

sys.path.insert(0, "/opt/trn_rl_repo")

import numpy as np

# ---- hardcoded geometry (from the problem spec) ----
Y = 64
XV = 50000
T = 1_000_000
NCORES = 8
TCORE = T // NCORES  # 125000

GRP = 2                 # independent sub-batches (PE<->DVE pipelining)
F = 112                 # chain-pairs per sub-batch
B = GRP * 2 * F         # 448 chains per core
L = 280                 # steps per chain; B*L = 125440 >= TCORE
WINDOWS = [96, 96, 88]
D = 8                   # steps per emission DMA batch
NPX = 16                # chain-pairs per output staging tile
BL = B * L              # padded output rows per core
WARM = 32               # host warmup steps

assert sum(WINDOWS) == L and B * L >= TCORE

LAST_RESULTS = None  # stashed BassKernelResults for test harness introspection

_CACHED_NC = None


def _build_bass():
    import concourse.tile as tile
    from concourse import bacc, mybir
    from contextlib import ExitStack

    f32 = mybir.dt.float32
    nc = bacc.Bacc("TRN2", target_bir_lowering=False)

    E = nc.dram_tensor("E", [GRP, 128, L, F], f32, kind="ExternalInput")
    # CONST = [AB (128) | identity (128) | V (GRP*F)] packed so the kernel
    # head issues a single DMA wait (LDWEIGHTS tolerates only one sync wait).
    CONST = nc.dram_tensor("CONST", [128, 256 + GRP * F], f32, kind="ExternalInput")
    OUT = nc.dram_tensor("OUT", [BL, 64], f32, kind="ExternalOutput")

    with tile.TileContext(nc) as tc, ExitStack() as ctx:
        singles = ctx.enter_context(tc.tile_pool(name="singles", bufs=1))
        hist_p = ctx.enter_context(tc.tile_pool(name="hist", bufs=2))
        e_p = ctx.enter_context(tc.tile_pool(name="ebuf", bufs=2))
        stag_p = ctx.enter_context(tc.tile_pool(name="stag", bufs=2))
        ps_rec = ctx.enter_context(tc.tile_pool(name="psrec", bufs=4, space="PSUM"))
        ps_tp = ctx.enter_context(tc.tile_pool(name="pstp", bufs=2, space="PSUM"))

        const_sb = singles.tile([128, 256 + GRP * F], f32)
        nc.sync.dma_start(const_sb[:], CONST[:])
        ab_sb = const_sb[:, 0:128]
        id_sb = const_sb[:, 128:256]

        # chain flat index c = (grp*2 + g)*F + f covers rows [c*L, (c+1)*L)
        out_r = OUT[:].rearrange("(grp g f l) j -> l grp g f j", grp=GRP, g=2, f=F)

        s_prev = [
            const_sb[:, 256 + grp * F : 256 + (grp + 1) * F] for grp in range(GRP)
        ]
        w0 = 0
        for kw in WINDOWS:
            hist = hist_p.tile([128, GRP, F, max(WINDOWS)], f32, tag="hist")
            for d0 in range(0, kw, D):
                dd = min(D, kw - d0)
                e_bufs = []
                for grp in range(GRP):
                    eb = e_p.tile([128, D, F], f32, tag=f"ebuf{grp}")
                    nc.sync.dma_start(
                        eb[:, :dd, :], E[grp, :, w0 + d0 : w0 + d0 + dd, :]
                    )
                    e_bufs.append(eb)
                for s in range(d0, d0 + dd):
                    for grp in range(GRP):
                        ps = ps_rec.tile([128, F], f32, tag="ps")
                        nc.tensor.matmul(ps[:], ab_sb, s_prev[grp])
                        nc.vector.tensor_mul(
                            out=hist[:, grp, :, s],
                            in0=ps[:],
                            in1=e_bufs[grp][:, s - d0, :],
                        )
                        s_prev[grp] = hist[:, grp, :, s]
            # output stage for this window (overlaps next window's recurrence)
            for grp in range(GRP):
                for f0 in range(0, F, NPX):
                    npx = min(NPX, F - f0)
                    stag = stag_p.tile([128, NPX, 128], f32, tag="stag")
                    for j4 in range(0, npx, 4):
                        n4 = min(4, npx - j4)
                        pt = ps_tp.tile([128, 4, 128], f32, tag="pt")
                        for j in range(n4):
                            f = f0 + j4 + j
                            nc.tensor.transpose(
                                pt[:kw, j, :], hist[:, grp, f, :kw], id_sb
                            )
                        nc.scalar.mul(
                            out=stag[:kw, j4 : j4 + n4, :],
                            in_=pt[:kw, :n4, :],
                            mul=1.0,
                        )
                    for g in range(2):
                        nc.sync.dma_start(
                            out_r[w0 : w0 + kw, grp, g, f0 : f0 + npx, :],
                            stag[:kw, :npx, g * 64 : (g + 1) * 64],
                        )
            w0 += kw
    nc.compile()
    return nc


def _prepare_inputs(x, transition, b, pi):
    """Host-side planning: emission pre-gather, chain seeds, constants."""
    A64 = transition.astype(np.float64)
    bs32 = (b * np.float32(XV)).astype(np.float32)  # prescaled emissions

    # pad x so padded chain tails index valid emissions
    pad = ((NCORES - 1) * TCORE + BL) - T  # = BL - TCORE
    x_pad = np.concatenate([x, np.repeat(x[-1:], pad)]).astype(np.int64)

    # ---- chain seeds: v_c ~ alpha_{start-1}; device step yields alpha_start ----
    starts = np.empty((NCORES, B), np.int64)
    for k in range(NCORES):
        starts[k] = k * TCORE + np.arange(B) * L
    flat_starts = starts.ravel()

    Vv = np.ones((NCORES * B, Y), np.float64) / Y
    warm_mask = flat_starts > 0
    widx = np.empty((warm_mask.sum(), WARM), np.int64)
    widx[:] = flat_starts[warm_mask, None] - WARM + np.arange(WARM)[None, :]
    bT64 = np.ascontiguousarray(b.astype(np.float64).T)  # (XV, Y)
    EW = bT64[x_pad[widx]]  # (M, WARM, Y)
    Vw = Vv[warm_mask]
    for s in range(WARM):
        Vw = (Vw @ A64) * EW[:, s, :]
        Vw /= Vw.sum(1, keepdims=True)
    Vv[warm_mask] = Vw
    # global chain 0: A^T v = pi  so that (v @ A) * e0 == pi * e0 exactly
    Vv[0] = np.linalg.solve(A64.T, pi.astype(np.float64))
    Vv = Vv.astype(np.float32).reshape(NCORES, B, Y)

    ABm = np.zeros((128, 128), np.float32)
    ABm[:64, :64] = transition.astype(np.float32)
    ABm[64:, 64:] = transition.astype(np.float32)
    Im = np.eye(128, dtype=np.float32)

    # ---- per-core emission streams:
    # E[grp, g*64+j, s, f] = bs[j, x[k*TCORE + c*L + s]],  c = (grp*2+g)*F + f
    in_maps = []
    for k in range(NCORES):
        idx = np.empty((B, L), np.int64)
        idx[:] = (k * TCORE + np.arange(B) * L)[:, None] + np.arange(L)[None, :]
        tok = x_pad[idx]  # (B, L) token ids
        Ek = np.empty((GRP, 128, L, F), np.float32)
        for grp in range(GRP):
            for g in range(2):
                c0 = (grp * 2 + g) * F
                tg = np.ascontiguousarray(tok[c0 : c0 + F].T)  # (L, F)
                np.take(
                    bs32,
                    tg.ravel(),
                    axis=1,
                    out=Ek[grp, g * 64 : (g + 1) * 64].reshape(64, L * F),
                )
        Ck = np.empty((128, 256 + GRP * F), np.float32)
        Ck[:, 0:128] = ABm
        Ck[:, 128:256] = Im
        for grp in range(GRP):
            for g in range(2):
                c0 = (grp * 2 + g) * F
                Ck[g * 64 : (g + 1) * 64, 256 + grp * F : 256 + (grp + 1) * F] = Vv[
                    k, c0 : c0 + F
                ].T
        in_maps.append({"E": Ek, "CONST": Ck})
    return in_maps


def kernel(x, transition, b, pi):
    global LAST_RESULTS, _CACHED_NC
    from concourse.bass_utils import run_bass_kernel_spmd

    in_maps = _prepare_inputs(
        np.asarray(x), np.asarray(transition), np.asarray(b), np.asarray(pi)
    )
    if _CACHED_NC is None:
        _CACHED_NC = _build_bass()
    res = run_bass_kernel_spmd(_CACHED_NC, in_maps, core_ids=list(range(NCORES)))
    LAST_RESULTS = res

    full = np.concatenate([r["OUT"][:TCORE] for r in res.results], axis=0)
    full = full / full.sum(axis=1, keepdims=True)
    return full.astype(np.float32)



# revision 6
# speedup vs baseline: 4.4609x; 4.4609x over previous
"""HMM scaled-forward (alpha scaling) kernel for Trainium2, 8 NeuronCores.

Math: alpha_t = normalize((alpha_{t-1} @ A) * b[:, x_t]).
The map v -> normalize((v @ A) * e) is a Hilbert-metric contraction (A is a
dense positive stochastic matrix; diagonal emission scaling is an isometry),
so the T=1M sequential scan is split into independent chains, each seeded by
a 32-step host-side warmup (converges to fp32 machine precision in ~16
steps). Per-step normalization is dropped on device (prescaled emissions
keep the unnormalized state within e^{+-6} over a 32-step chain); rows are
normalized on the host at the end.

v2 layout (vs the v1 transpose-on-PE design):
  - Everything on device is bf16 (rel-err budget 2e-2; bf16 adds ~0.3%):
    4x faster matmuls (1 cycle/row vs 4), FWL weight loads, half the DMA.
  - No on-chip transposes: history is DMA'd out in [state, chain, step]
    layout and the host reassembles the (T, 64) output.
  - Per step and group: PE matmul (state @ blockdiag(A,A) -> PSUM fp32),
    ACT copies PSUM -> SBUF bf16, DVE multiplies by emissions in 2x_1P.
  - G=4 independent groups pipeline PE->ACT->DVE; F=490 chain-pairs per
    group (2x490 chains in 128 partitions); L=32 steps per chain.
Per-core totals: 128 matmuls (N=490), 128 ACT copies, 128 DVE multiplies,
4+4 windowed 4MB DMAs (in: emissions, out: history), all double-buffered.
"""

import sys

sys.path.insert(0, "/opt/trn_rl_repo")

import numpy as np

# ---- hardcoded geometry (from the problem spec) ----
Y = 64
XV = 50000
T = 1_000_000
NCORES = 8
TCORE = T // NCORES  # 125000

G = 5                   # independent groups (PE<->ACT<->DVE pipelining)
F = 392                 # chain-pairs per group (PSUM bank: 392*4B < 2KB)
B = G * 2 * F           # 3920 chains per core
L = 32                  # steps per chain; B*L = 125440 >= TCORE
W = 4                   # steps per window (DMA batch); L % W == 0
NW = L // W
BL = B * L              # padded output rows per core
WARM = 32               # host warmup steps
HPATCH = 16             # leading output rows recomputed exactly on the host

assert B * L >= TCORE and L % W == 0

LAST_RESULTS = None  # stashed BassKernelResults for test harness introspection

_CACHED_NC = None


def _build_bass():
    import concourse.tile as tile
    from concourse import bacc, mybir
    from contextlib import ExitStack

    bf16 = mybir.dt.bfloat16
    f32 = mybir.dt.float32
    nc = bacc.Bacc("TRN2", target_bir_lowering=False)

    E = nc.dram_tensor("E", [128, G, L, F], bf16, kind="ExternalInput")
    # CONST = [AB (128 cols) | seeds (G*F cols)] packed so the kernel head
    # issues a single DMA wait (LDWEIGHTS tolerates only one sync wait).
    CONST = nc.dram_tensor("CONST", [128, 128 + G * F], bf16, kind="ExternalInput")
    OUT = nc.dram_tensor("OUT", [128, G, L, F], bf16, kind="ExternalOutput")

    with tile.TileContext(nc) as tc, ExitStack() as ctx:
        singles = ctx.enter_context(tc.tile_pool(name="singles", bufs=1))
        hist_p = ctx.enter_context(tc.tile_pool(name="hist", bufs=2))
        e_p = ctx.enter_context(tc.tile_pool(name="ebuf", bufs=2))
        pbuf_p = ctx.enter_context(tc.tile_pool(name="pbuf", bufs=8))
        ps_rec = ctx.enter_context(tc.tile_pool(name="psrec", bufs=8, space="PSUM"))

        const_sb = singles.tile([128, 128 + G * F], bf16)
        nc.sync.dma_start(const_sb[:], CONST[:])
        ab_sb = const_sb[:, 0:128]

        s_prev = [const_sb[:, 128 + g * F : 128 + (g + 1) * F] for g in range(G)]

        for w in range(NW):
            eb = e_p.tile([128, G, W, F], bf16, tag="ebuf")
            nc.sync.dma_start(eb[:], E[:, :, w * W : (w + 1) * W, :])
            hist = hist_p.tile([128, G, W, F], bf16, tag="hist")
            for s in range(W):
                for g in range(G):
                    ps = ps_rec.tile([128, F], f32, tag="ps")
                    nc.tensor.matmul(ps[:], ab_sb, s_prev[g])
                    pb = pbuf_p.tile([128, F], bf16, tag="pb")
                    nc.scalar.copy(out=pb[:], in_=ps[:])
                    nc.vector.tensor_mul(
                        out=hist[:, g, s, :],
                        in0=pb[:],
                        in1=eb[:, g, s, :],
                    )
                    s_prev[g] = hist[:, g, s, :]
            nc.sync.dma_start(OUT[:, :, w * W : (w + 1) * W, :], hist[:])
    nc.compile()
    return nc


def _prepare_inputs(x, transition, b, pi):
    """Host-side planning: emission pre-gather, chain seeds, constants."""
    import ml_dtypes

    bft = ml_dtypes.bfloat16
    A32 = transition.astype(np.float32)
    bs32 = (b * np.float32(XV)).astype(np.float32)  # prescaled emissions
    bs16 = bs32.astype(bft)

    # pad x so padded chain tails index valid emissions
    pad = ((NCORES - 1) * TCORE + BL) - T  # = BL - TCORE
    x_pad = np.concatenate([x, np.repeat(x[-1:], pad)]).astype(np.int64)

    # ---- chain seeds: v_c ~ alpha_{start-1}; device step yields alpha_start ----
    starts = np.empty((NCORES, B), np.int64)
    for k in range(NCORES):
        starts[k] = k * TCORE + np.arange(B) * L
    flat_starts = starts.ravel()

    Vv = np.ones((NCORES * B, Y), np.float32) / Y
    warm_mask = flat_starts > 0
    widx = np.empty((warm_mask.sum(), WARM), np.int64)
    widx[:] = flat_starts[warm_mask, None] - WARM + np.arange(WARM)[None, :]
    bT32 = np.ascontiguousarray(b.astype(np.float32).T)  # (XV, Y)
    EW = bT32[x_pad[widx]]  # (M, WARM, Y)
    Vw = Vv[warm_mask]
    for s in range(WARM):
        Vw = (Vw @ A32) * EW[:, s, :]
        Vw /= Vw.sum(1, keepdims=True)
    Vv[warm_mask] = Vw
    # global chain 0 has no true predecessor: seed with pi; its first HPATCH
    # rows are recomputed exactly on the host (contraction makes the rest
    # converge well before row HPATCH).
    Vv[0] = pi.astype(np.float32)
    Vv = Vv.reshape(NCORES, B, Y)

    ABm = np.zeros((128, 128), np.float32)
    ABm[:64, :64] = A32
    ABm[64:, 64:] = A32

    # ---- per-core emission streams:
    # E[h*64+j, g, s, f] = bs[j, x[k*TCORE + c*L + s]],  c = (g*2+h)*F + f
    in_maps = []
    for k in range(NCORES):
        Ek = np.empty((128, G, L, F), bft)
        for g in range(G):
            for h in range(2):
                c0 = (g * 2 + h) * F
                idx = np.empty((F, L), np.int64)
                idx[:] = (k * TCORE + (c0 + np.arange(F)) * L)[:, None] + np.arange(L)[
                    None, :
                ]
                tok = np.ascontiguousarray(x_pad[idx].T)  # (L, F)
                Ek[h * 64 : (h + 1) * 64, g] = np.take(
                    bs16, tok.ravel(), axis=1
                ).reshape(64, L, F)
        Ck = np.empty((128, 128 + G * F), np.float32)
        Ck[:, 0:128] = ABm
        for g in range(G):
            for h in range(2):
                c0 = (g * 2 + h) * F
                Ck[h * 64 : (h + 1) * 64, 128 + g * F : 128 + (g + 1) * F] = Vv[
                    k, c0 : c0 + F
                ].T
        in_maps.append({"E": Ek, "CONST": Ck.astype(bft)})
    return in_maps


def kernel(x, transition, b, pi):
    global LAST_RESULTS, _CACHED_NC
    from concourse.bass_utils import run_bass_kernel_spmd

    in_maps = _prepare_inputs(
        np.asarray(x), np.asarray(transition), np.asarray(b), np.asarray(pi)
    )
    if _CACHED_NC is None:
        _CACHED_NC = _build_bass()
    res = run_bass_kernel_spmd(_CACHED_NC, in_maps, core_ids=list(range(NCORES)))
    LAST_RESULTS = res

    # decode: OUT[h*64+j, g, s, f] -> row (c*L + s, j), c = (g*2+h)*F + f
    blocks = []
    for k in range(NCORES):
        o = res.results[k]["OUT"].astype(np.float32)  # (128, G, L, F)
        o = o.reshape(2, 64, G, L, F)  # (h, j, g, s, f)
        o = o.transpose(2, 0, 4, 3, 1)  # (g, h, f, s, j)
        blocks.append(o.reshape(BL, Y)[:TCORE])
    full = np.concatenate(blocks, axis=0)
    full = full / full.sum(axis=1, keepdims=True)

    # exact fp64 recurrence for the first HPATCH rows (chain 0 has no
    # converged predecessor to warm up from)
    x = np.asarray(x)
    A64 = np.asarray(transition).astype(np.float64)
    b64 = np.asarray(b).astype(np.float64)
    a = b64[:, x[0]] * np.asarray(pi).astype(np.float64)
    a /= a.sum()
    full[0] = a
    for t in range(1, HPATCH):
        a = (a @ A64) * b64[:, x[t]]
        a /= a.sum()
        full[t] = a
    return full.astype(np.float32)


# revision 8
# speedup vs baseline: 4.7211x; 1.0583x over previous
"""HMM scaled-forward (alpha scaling) kernel for Trainium2, 8 NeuronCores.

Math: alpha_t = normalize((alpha_{t-1} @ A) * b[:, x_t]).
The map v -> normalize((v @ A) * e) is a Hilbert-metric contraction (A is a
dense positive stochastic matrix; diagonal emission scaling is an isometry),
so the T=1M sequential scan is split into independent chains, each seeded by
a 32-step host-side warmup. Per-step normalization is dropped on device
(quantized emissions + 1/qmean-scaled transition keep the unnormalized state
within e^{+-3} over a 32-step chain); rows are normalized on the host.

Device design (memory-bound problem: ~25MB HBM traffic per core):
  - Emissions are pre-gathered on the host (TRN2 has no fast dynamic
    gather), quantized to uint8 with one global scale (values are ~2*U[0,1]
    after column normalization, so every column max stays within 0.25%),
    and cast uint8->bf16 during the SWDGE DMA: 8.3MB in per core.
  - History is written back as bf16 in [state, chain, step] window-major
    layout (one contiguous run per partition per window DMA): 16.5MB out.
  - Recurrence per step and group: PE matmul (bf16, N=392) -> PSUM fp32;
    PSUM->SBUF bf16 copy split ACT(4/5)/DVE(1/5); emission multiply in
    bf16 2x split DVE(2/3)/GPSIMD(1/3). G=5 groups pipeline the engines.
  - The (T, 64) output is reassembled / normalized on the host.
"""

import sys

sys.path.insert(0, "/opt/trn_rl_repo")

import numpy as np

# ---- hardcoded geometry (from the problem spec) ----
Y = 64
XV = 50000
T = 1_000_000
NCORES = 8
TCORE = T // NCORES  # 125000

G = 5                   # independent groups (PE/ACT/DVE/GPSIMD pipelining)
F = 392                 # chain-pairs per group (PSUM bank: 392*4B < 2KB)
B = G * 2 * F           # 3920 chains per core
L = 32                  # steps per chain; B*L = 125440 >= TCORE
W = 4                   # steps per window (DMA batch); L % W == 0
NW = L // W
BL = B * L              # padded output rows per core
WARM = 32               # host warmup steps
HPATCH = 16             # leading output rows recomputed exactly on the host

assert B * L >= TCORE and L % W == 0

LAST_RESULTS = None  # stashed BassKernelResults for test harness introspection

_CACHED_NC = None


def _build_bass():
    import concourse.tile as tile
    from concourse import bacc, mybir
    from contextlib import ExitStack

    bf16 = mybir.dt.bfloat16
    f32 = mybir.dt.float32
    u8 = mybir.dt.uint8
    nc = bacc.Bacc("TRN2", target_bir_lowering=False)

    # window-major so each window transfer is one contiguous run/partition
    E = nc.dram_tensor("E", [128, NW, G, W, F], u8, kind="ExternalInput")
    # CONST = [AB (128 cols) | seeds (G*F cols)] packed so the kernel head
    # issues a single DMA wait (LDWEIGHTS tolerates only one sync wait).
    CONST = nc.dram_tensor("CONST", [128, 128 + G * F], bf16, kind="ExternalInput")
    OUT = nc.dram_tensor("OUT", [128, NW, G, W, F], bf16, kind="ExternalOutput")

    with tile.TileContext(nc) as tc, ExitStack() as ctx:
        singles = ctx.enter_context(tc.tile_pool(name="singles", bufs=1))
        hist_p = ctx.enter_context(tc.tile_pool(name="hist", bufs=2))
        e_p = ctx.enter_context(tc.tile_pool(name="ebuf", bufs=2))
        pbuf_p = ctx.enter_context(tc.tile_pool(name="pbuf", bufs=8))
        ps_rec = ctx.enter_context(tc.tile_pool(name="psrec", bufs=8, space="PSUM"))

        const_sb = singles.tile([128, 128 + G * F], bf16)
        nc.sync.dma_start(const_sb[:], CONST[:])
        ab_sb = const_sb[:, 0:128]

        s_prev = [const_sb[:, 128 + g * F : 128 + (g + 1) * F] for g in range(G)]

        cctr = 0  # copy-split counter
        mctr = 0  # multiply-split counter
        for w in range(NW):
            eb = e_p.tile([128, G, W, F], bf16, tag="ebuf")
            nc.gpsimd.dma_start(eb[:], E[:, w])  # SWDGE casts u8 -> bf16
            hist = hist_p.tile([128, G, W, F], bf16, tag="hist")
            for s in range(W):
                for g in range(G):
                    ps = ps_rec.tile([128, F], f32, tag="ps")
                    nc.tensor.matmul(ps[:], ab_sb, s_prev[g])
                    pb = pbuf_p.tile([128, F], bf16, tag="pb")
                    # PSUM->SBUF bf16 copy: only ACT/DVE can read PSUM
                    if cctr % 5 == 4:
                        nc.vector.tensor_copy(out=pb[:], in_=ps[:])
                    else:
                        nc.scalar.copy(out=pb[:], in_=ps[:])
                    cctr += 1
                    # bf16 emission multiply (2x on DVE); GPSIMD takes 1/3
                    if mctr % 3 == 2:
                        eng = nc.gpsimd
                    else:
                        eng = nc.vector
                    eng.tensor_mul(
                        out=hist[:, g, s, :],
                        in0=pb[:],
                        in1=eb[:, g, s, :],
                    )
                    mctr += 1
                    s_prev[g] = hist[:, g, s, :]
            nc.sync.dma_start(OUT[:, w], hist[:])
    nc.compile()
    return nc


def _prepare_inputs(x, transition, b, pi):
    """Host-side planning: emission pre-gather + uint8 quantization, chain
    seeds, constants."""
    import ml_dtypes

    bft = ml_dtypes.bfloat16
    A32 = transition.astype(np.float32)

    # global-scale uint8 quantization of the emission matrix
    bmax = float(b.max())
    bq = np.clip(np.rint(b * (255.0 / bmax)), 0, 255).astype(np.uint8)
    qmean = float(bq.mean())

    # pad x so padded chain tails index valid emissions
    pad = ((NCORES - 1) * TCORE + BL) - T  # = BL - TCORE
    x_pad = np.concatenate([x, np.repeat(x[-1:], pad)]).astype(np.int64)

    # ---- chain seeds: v_c ~ alpha_{start-1}; device step yields alpha_start ----
    starts = np.empty((NCORES, B), np.int64)
    for k in range(NCORES):
        starts[k] = k * TCORE + np.arange(B) * L
    flat_starts = starts.ravel()

    Vv = np.ones((NCORES * B, Y), np.float32) / Y
    warm_mask = flat_starts > 0
    widx = np.empty((warm_mask.sum(), WARM), np.int64)
    widx[:] = flat_starts[warm_mask, None] - WARM + np.arange(WARM)[None, :]
    bT32 = np.ascontiguousarray(b.astype(np.float32).T)  # (XV, Y)
    EW = bT32[x_pad[widx]]  # (M, WARM, Y)
    Vw = Vv[warm_mask]
    for s in range(WARM):
        Vw = (Vw @ A32) * EW[:, s, :]
        Vw /= Vw.sum(1, keepdims=True)
    Vv[warm_mask] = Vw
    # global chain 0 has no true predecessor: seed with pi; its first HPATCH
    # rows are recomputed exactly on the host (contraction makes the rest
    # converge well before row HPATCH).
    Vv[0] = pi.astype(np.float32)
    Vv = Vv.reshape(NCORES, B, Y)

    # transition scaled by 1/qmean so the unnormalized state stays O(1)
    ABm = np.zeros((128, 128), np.float32)
    ABm[:64, :64] = A32 / qmean
    ABm[64:, 64:] = A32 / qmean

    # ---- per-core emission streams:
    # E[h*64+j, w, g, s, f] = bq[j, x[k*TCORE + c*L + w*W + s]], c=(g*2+h)*F+f
    in_maps = []
    for k in range(NCORES):
        Ek = np.empty((128, NW, G, W, F), np.uint8)
        for g in range(G):
            for h in range(2):
                c0 = (g * 2 + h) * F
                idx = np.empty((F, L), np.int64)
                idx[:] = (k * TCORE + (c0 + np.arange(F)) * L)[:, None] + np.arange(L)[
                    None, :
                ]
                tok = np.ascontiguousarray(x_pad[idx].T)  # (L, F)
                Ek[h * 64 : (h + 1) * 64, :, g] = np.take(
                    bq, tok.ravel(), axis=1
                ).reshape(64, NW, W, F)
        Ck = np.empty((128, 128 + G * F), np.float32)
        Ck[:, 0:128] = ABm
        for g in range(G):
            for h in range(2):
                c0 = (g * 2 + h) * F
                Ck[h * 64 : (h + 1) * 64, 128 + g * F : 128 + (g + 1) * F] = Vv[
                    k, c0 : c0 + F
                ].T
        in_maps.append({"E": Ek, "CONST": Ck.astype(bft)})
    return in_maps


def kernel(x, transition, b, pi):
    global LAST_RESULTS, _CACHED_NC
    from concourse.bass_utils import run_bass_kernel_spmd

    x = np.asarray(x)
    transition = np.asarray(transition)
    b = np.asarray(b)
    pi = np.asarray(pi)
    in_maps = _prepare_inputs(x, transition, b, pi)
    if _CACHED_NC is None:
        _CACHED_NC = _build_bass()
    res = run_bass_kernel_spmd(_CACHED_NC, in_maps, core_ids=list(range(NCORES)))
    LAST_RESULTS = res

    # decode: OUT[h*64+j, w, g, s, f] -> row (c*L + w*W + s, j), c=(g*2+h)*F+f
    blocks = []
    for k in range(NCORES):
        o = res.results[k]["OUT"].astype(np.float32)  # (128, NW, G, W, F)
        o = o.reshape(2, 64, NW, G, W, F)  # (h, j, w, g, s, f)
        o = o.transpose(3, 0, 5, 2, 4, 1)  # (g, h, f, w, s, j)
        blocks.append(o.reshape(BL, Y)[:TCORE])
    full = np.concatenate(blocks, axis=0)
    full = full / full.sum(axis=1, keepdims=True)

    # exact fp64 recurrence for the first HPATCH rows (chain 0 has no
    # converged predecessor to warm up from)
    A64 = transition.astype(np.float64)
    b64 = b.astype(np.float64)
    a = b64[:, x[0]] * pi.astype(np.float64)
    a /= a.sum()
    full[0] = a
    for t in range(1, HPATCH):
        a = (a @ A64) * b64[:, x[t]]
        a /= a.sum()
        full[t] = a
    return full.astype(np.float32)
